# revision 1
# baseline (speedup 1.0000x reference)
"""Additive (Bahdanau) attention on 8 trn2 NeuronCores.

Math (per batch element b, handled by one core):
  q_[tq, a]   = query[tq, :] @ Wq[a, :]          (A = 128 attention dim)
  k_[tk, a]   = key[tk, :]   @ Wk[a, :]
  logits[q,k] = sum_a w_w[a] * tanh(q_[q,a] + k_[k,a] + bias[a])   (+ w_b, which
                cancels in softmax and is therefore skipped)
  attn        = softmax_k(logits)
  out         = attn @ value

Mapping (one batch element per core, 8 cores):
  - A=128 lives on partitions for the tanh stage:
      k_pb  [a=128, TK=512]  (k projection + bias),  q_pT [a=128, TQ=512]
  - per query q the tanh argument k_pb + q_pT[:, q] is built either by a DVE
    tensor_scalar_add into a block buffer followed by one big ScalarE Tanh
    (batched blocks), or fused directly into the ScalarE activation as a
    per-partition bias (fused blocks). The split balances DVE vs ScalarE.
  - tanh output is bf16; the PE reduces over A with a shifting zero-padded
    bf16 w_w window as the stationary operand:
      matmul(lhsT = wbuf[:, 128-col:256-col], rhs = tanh_block)
    accumulating logits^T [128 q, 512 k] into one psum bank per 128-query
    group (query col = psum partition).
  - the four query groups are processed INTERLEAVED so the four matmuls for
    the same col share one weight window: a post-compile pass deletes the
    redundant LDWEIGHTS (weights persist in the PE array), letting the four
    matmuls run back-to-back at stream rate.
  - epilogue per group: Exp with accum_out rowsums (softmax without
    max-subtraction: logits ~ N(0, ~0.8), exp exact to 2ulp on [-10, 10]),
    DVE reciprocal + scale -> attn rows; PE transpose + matmul against value.

Sync-wait discipline: each trn2 instruction encodes one semaphore wait; bacc
legalizes more via event-semaphore chains, but each chain costs ~100ns+, so
slot-recycling hazards are carried by a single designated instruction per
block and the transitively-implied duplicate edges are demoted to no-sync
(ordering-only) edges.
"""

import numpy as np
import ml_dtypes

import concourse.bass as bass
import concourse.tile as tile
from concourse import bacc, mybir
from concourse.bass_utils import run_bass_kernel_spmd

F32 = mybir.dt.float32
BF16 = mybir.dt.bfloat16
AF = mybir.ActivationFunctionType

B, TQ, TK, DQ, DK, DV, A = 8, 512, 512, 512, 512, 512, 128
NB = 16          # queries per block
N_GROUPS = TQ // 128
BLOCKS_PER_GROUP = 128 // NB
N_SUMS_BUFS = 3
N_HID_BUFS = 3
# fused-ACT blocks per 128-query group (rest are DVE+batched-tanh);
# chosen to balance ScalarE vs VectorE busy time
FUSED_BLOCKS = [1, 0, 1, 0]
N_CORES = 8

_CACHE = {}

_add_dep_helper = bass._add_dep_helper
_DEP_SYNC = mybir.DependencyInfo.SYNC_ONLY
_DEP_NOSYNC = mybir.DependencyInfo.NO_SYNC_ONLY


def _demote(ins, dep_ins):
    """Demote a sync dependency edge to a no-sync (ordering-only) edge."""
    if ins.try_remove_dependency(dep_ins.name, _DEP_SYNC):
        ins.add_dependency(dep_ins.name, _DEP_NOSYNC)


def _dedup_ldweights(nc):
    """Remove LDWEIGHTS whose weights AP equals the immediately preceding
    LDWEIGHTS' (the PE array keeps the stationary operand between matmuls).
    Only wait-free, update-free LDWs are removed."""
    removed = 0
    for f in nc.m.functions:
        for bb in f.blocks:
            keep = []
            last_sig = None
            for ins in bb.instructions:
                if type(ins).__name__ == "InstLdweights":
                    sig = str(ins.ins[0])
                    si = ins.sync_info
                    clean = si is None or (not si.on_wait and not si.on_update)
                    if sig == last_sig and clean:
                        removed += 1
                        continue
                    last_sig = sig
                keep.append(ins)
            if len(keep) != len(bb.instructions):
                bb.instructions = keep
    return removed


def build_nc():
    nc = bacc.Bacc(None, target_bir_lowering=False, debug=False)

    qT = nc.declare_dram_parameter("qT", [DQ, TQ], F32, isOutput=False)
    kT = nc.declare_dram_parameter("kT", [DK, TK], F32, isOutput=False)
    val = nc.declare_dram_parameter("value", [TK, DV], F32, isOutput=False)
    WqT = nc.declare_dram_parameter("WqT", [DQ, A], F32, isOutput=False)
    WkT = nc.declare_dram_parameter("WkT", [DK, A], F32, isOutput=False)
    # column 0: additive bias; column 1: zeros (used as AP bias for Tanh/Exp
    # so bass does not materialize a const-AP, which would cost extra waits)
    bias = nc.declare_dram_parameter("bias", [A, 2], F32, isOutput=False)
    # host-precomputed: zeros with w_w in column 128 (matvec weight window)
    wbuf_d = nc.declare_dram_parameter("wbuf", [128, 256], BF16, isOutput=False)
    # host-precomputed 128x128 identity (PE transpose operand)
    ident_d = nc.declare_dram_parameter("ident", [128, 128], F32, isOutput=False)
    attn_out = nc.declare_dram_parameter("attn", [TQ, TK], F32, isOutput=True)
    out_out = nc.declare_dram_parameter("out", [TQ, DV], F32, isOutput=True)

    with tile.TileContext(nc) as tc:
        with (
            tc.tile_pool(name="pers", bufs=1) as pers,
            tc.tile_pool(name="stage", bufs=1) as stage,
            tc.tile_pool(name="sums", bufs=N_SUMS_BUFS) as sums_pool,
            tc.tile_pool(name="hid", bufs=N_HID_BUFS) as hid_pool,
            tc.tile_pool(name="epi", bufs=4) as epi_pool,
            tc.tile_pool(name="small", bufs=4) as small_pool,
            tc.tile_pool(name="attnT", bufs=2) as attnT_pool,
            tc.tile_pool(name="plog", bufs=2, space="PSUM") as plog_pool,
            tc.tile_pool(name="pout", bufs=2, space="PSUM") as pout_pool,
            tc.tile_pool(name="ptr", bufs=2, space="PSUM") as ptr_pool,
            tc.tile_pool(name="pobs", bufs=1, space="PSUM") as pobs_pool,
        ):
            # ---- persistent tiles ----
            value_sb = pers.tile([128, TK // 128, DV], F32)
            q_pT = pers.tile([128, TQ], F32)
            k_pb = pers.tile([128, TK], F32)
            wbuf = pers.tile([128, 256], BF16)
            bias_sb = pers.tile([128, 2], F32)
            ident = pers.tile([128, 128], F32)
            scr11 = pers.tile([1, 1], F32)

            nc.sync.dma_start(out=wbuf[:], in_=wbuf_d[:, :])
            nc.sync.dma_start(out=ident[:], in_=ident_d[:, :])
            nc.sync.dma_start(out=bias_sb[:, :], in_=bias[:, :])
            nc.sync.dma_start(
                out=value_sb[:], in_=val.rearrange("(c p) d -> p c d", p=128)
            )

            # ---- staged inputs for the projections ----
            WqT_sb = stage.tile([128, DQ // 128, A], F32)
            WkT_sb = stage.tile([128, DK // 128, A], F32)
            qT_sb = stage.tile([128, DQ // 128, TQ], F32)
            kT_sb = stage.tile([128, DK // 128, TK], F32)
            # k/q staging; k first (every block depends on k_pb), and kT in
            # two chunks so the projection matmuls pipeline with the DMA
            nc.sync.dma_start(out=WkT_sb[:], in_=WkT.rearrange("(c p) a -> p c a", p=128))
            kT_re = kT.rearrange("(c p) t -> p c t", p=128)
            nc.sync.dma_start(out=kT_sb[:, 0:2, :], in_=kT_re[:, 0:2, :])
            nc.sync.dma_start(out=kT_sb[:, 2:4, :], in_=kT_re[:, 2:4, :])
            nc.sync.dma_start(out=WqT_sb[:], in_=WqT.rearrange("(c p) a -> p c a", p=128))
            nc.sync.dma_start(out=qT_sb[:], in_=qT.rearrange("(c p) t -> p c t", p=128))

            # ---- PE observers: absorb one foreign semaphore each so that no
            # later matmul needs more than one sync wait ----
            obs = pobs_pool.tile([128, 1], F32)
            for src in (
                WqT_sb[:, 0, :],
                WkT_sb[:, 0, :],
                qT_sb[:, 0, 0:128],
                kT_sb[:, 0, 0:128],
                value_sb[:, 0, 0:128],
                ident[:, :],
            ):
                nc.tensor.matmul(obs[:], src, src[:, 0:1], start=True, stop=True)
            nc.tensor.matmul(
                obs[:], wbuf[:, 0:128], wbuf[:, 0:1], start=True, stop=True
            )
            # ACT observer for the bias DMA
            nc.scalar.copy(scr11[:], bias_sb[0:1, 0:1])
            zbias = bias_sb[:, 1:2]

            # ---- projections: k_pb[a, tk] first, then q_pT[a, tq] in two
            # chunks so the first blocks can start while the rest projects
            k_ps = plog_pool.tile([128, TK], F32, tag="plog")
            for c in range(DK // 128):
                nc.tensor.matmul(
                    k_ps[:], WkT_sb[:, c, :], kT_sb[:, c, :],
                    start=(c == 0), stop=(c == DK // 128 - 1),
                )
            nc.scalar.activation(
                k_pb[:], k_ps[:], AF.Identity, bias=bias_sb[:, 0:1], scale=1.0
            )

            Q_EARLY = 64
            q_ps = plog_pool.tile([128, TQ], F32, tag="plog")
            for c in range(DQ // 128):
                nc.tensor.matmul(
                    q_ps[:, 0:Q_EARLY], WqT_sb[:, c, :], qT_sb[:, c, 0:Q_EARLY],
                    start=(c == 0), stop=(c == DQ // 128 - 1),
                )
            nc.scalar.copy(q_pT[:, 0:Q_EARLY], q_ps[:, 0:Q_EARLY])
            for c in range(DQ // 128):
                nc.tensor.matmul(
                    q_ps[:, Q_EARLY:], WqT_sb[:, c, :], qT_sb[:, c, Q_EARLY:],
                    start=(c == 0), stop=(c == DQ // 128 - 1),
                )
            nc.scalar.copy(q_pT[:, Q_EARLY:], q_ps[:, Q_EARLY:])

            # ---- main loop ----
            def emit_epilogue(qbase, nrows, plog_ap):
                expt = epi_pool.tile([nrows, TK], F32)
                rowsum = small_pool.tile([nrows, 1], F32)
                nc.scalar.activation(
                    expt[:], plog_ap, AF.Exp, bias=zbias[0:nrows, :],
                    accum_out=rowsum[:, :],
                )
                recip = small_pool.tile([nrows, 1], F32)
                nc.vector.reciprocal(recip[:, :], rowsum[:, :])
                attn_sb = epi_pool.tile([nrows, TK], F32)
                nc.vector.tensor_scalar_mul(attn_sb[:], expt[:], recip[:, 0:1])
                nc.sync.dma_start(
                    out=attn_out[qbase:qbase + nrows, :], in_=attn_sb[:]
                )
                # out-path works on the UNNORMALIZED exp (parallel with the
                # attn normalize/DMA above); softmax normalization is folded
                # into the psum->sbuf copy as a per-partition scale
                out_ps = pout_pool.tile([nrows, DV], F32)
                for c in range(TK // 128):
                    tp = ptr_pool.tile([128, 128], F32)
                    nc.tensor.transpose(
                        tp[:, 0:nrows], expt[:, c * 128:(c + 1) * 128],
                        ident[0:nrows, 0:nrows],
                    )
                    attnT = attnT_pool.tile([128, 128], F32)
                    nc.vector.tensor_copy(attnT[:, 0:nrows], tp[:, 0:nrows])
                    nc.tensor.matmul(
                        out_ps[:], attnT[:, 0:nrows], value_sb[:, c, :],
                        start=(c == 0), stop=(c == TK // 128 - 1),
                    )
                out_sb = epi_pool.tile([nrows, DV], F32)
                nc.vector.tensor_scalar_mul(out_sb[:], out_ps[:], recip[:, 0:1])
                nc.sync.dma_start(
                    out=out_out[qbase:qbase + nrows, :], in_=out_sb[:]
                )

            sums_alloc = []      # allocation-order list of add-inst lists
            hid_alloc = []       # allocation order: (writer_insts, mm_insts)
            for g in range(N_GROUPS):
                plog = plog_pool.tile([128, TK], F32, tag="plog")
                n_fused = FUSED_BLOCKS[g]
                for blk in range(BLOCKS_PER_GROUP):
                    fused = blk >= BLOCKS_PER_GROUP - n_fused
                    aidx = len(hid_alloc)
                    hid = hid_pool.tile([128, NB * TK], BF16)
                    old = hid_alloc[aidx - N_HID_BUFS] if aidx >= N_HID_BUFS else None
                    writers = []
                    if not fused:
                        sums = sums_pool.tile([128, NB * TK], F32)
                        adds = []
                        for jj in range(NB):
                            q = g * 128 + blk * NB + jj
                            h = nc.vector.tensor_scalar_add(
                                sums[:, jj * TK:(jj + 1) * TK],
                                k_pb[:], q_pT[:, q:q + 1],
                            )
                            adds.append(h.ins)
                            if len(sums_alloc) >= N_SUMS_BUFS:
                                for o in sums_alloc[-N_SUMS_BUFS]:
                                    _demote(h.ins, o)
                        sums_alloc.append(adds)
                        if old is not None:
                            _add_dep_helper(
                                adds[1], old[1][-1], sync=True,
                                reason="hid slot recycle fence",
                            )
                        th = nc.scalar.activation(
                            hid[:], sums[:], AF.Tanh, bias=zbias
                        )
                        writers.append(th.ins)
                        if old is not None:
                            for wi in old[0]:
                                _demote(th.ins, wi)
                            for mi in old[1]:
                                _demote(th.ins, mi)
                    else:
                        for jj in range(NB):
                            q = g * 128 + blk * NB + jj
                            th = nc.scalar.activation(
                                hid[:, jj * TK:(jj + 1) * TK], k_pb[:],
                                AF.Tanh, bias=q_pT[:, q:q + 1],
                            )
                            if old is not None:
                                for wi in old[0]:
                                    _demote(th.ins, wi)
                                if jj > 0:
                                    for mi in old[1]:
                                        _demote(th.ins, mi)
                            writers.append(th.ins)
                    mms = []
                    for jj in range(NB):
                        col = blk * NB + jj
                        m = nc.tensor.matmul(
                            plog[:],
                            wbuf[:, 128 - col:256 - col],
                            hid[:, jj * TK:(jj + 1) * TK],
                            start=(col == 0), stop=(col == 127),
                        )
                        mms.append(m.ins)
                    hid_alloc.append((writers, mms))

                # ---- epilogue (overlaps the next group's main loop) ----
                emit_epilogue(g * 128, 128, plog[:])

    nc.compile()
    _dedup_ldweights(nc)
    return nc


def _get_nc():
    if "nc" not in _CACHE:
        _CACHE["nc"] = build_nc()
    return _CACHE["nc"]


def make_in_maps(query, key, value, Wq, Wk, bias, w_w, **_):
    WqT = np.ascontiguousarray(Wq.T, dtype=np.float32)
    WkT = np.ascontiguousarray(Wk.T, dtype=np.float32)
    bias_c = np.zeros((A, 2), dtype=np.float32)
    bias_c[:, 0] = np.asarray(bias, dtype=np.float32).reshape(A)
    wbuf_np = np.zeros((128, 256), dtype=np.float32)
    wbuf_np[:, 128] = np.asarray(w_w, dtype=np.float32).reshape(A)
    wbuf_np = wbuf_np.astype(ml_dtypes.bfloat16)
    ident_np = np.eye(128, dtype=np.float32)
    in_maps = []
    for b in range(B):
        in_maps.append({
            "qT": np.ascontiguousarray(query[b].T, dtype=np.float32),
            "kT": np.ascontiguousarray(key[b].T, dtype=np.float32),
            "value": np.ascontiguousarray(value[b], dtype=np.float32),
            "WqT": WqT,
            "WkT": WkT,
            "bias": bias_c,
            "wbuf": wbuf_np,
            "ident": ident_np,
        })
    return in_maps


def run(inputs, trace=False, **kwargs):
    nc = _get_nc()
    in_maps = make_in_maps(**{k: np.asarray(v) for k, v in inputs.items()})
    res = run_bass_kernel_spmd(
        nc, in_maps, list(range(N_CORES)), trace=trace, **kwargs
    )
    output = np.stack([res.results[b]["out"] for b in range(B)])
    attn = np.stack([res.results[b]["attn"] for b in range(B)])
    return (output, attn), res


def kernel(**inputs):
    (output, attn), _ = run(inputs)
    return output, attn

